# revision 51
# baseline (speedup 1.0000x reference)
"""Trainium2 Bass kernel for the ContextualActorSpike network.

Network (per reference): x = concat(obs, ctx) [B,192] broadcast over T=4 steps;
3x (Linear -> LIF) with HID=1024; feat = mean_t(spikes3); action_mean =
tanh(feat @ Wm.T + bm); action = action_mean + clip(noise, -.1, .1).

Strategy:
- Data-parallel over 8 NeuronCores: batch 16384 -> 2048 per core; weights
  replicated. No collectives.
- On-chip layout is [feature, batch]: activations live as 8 partition-tiles of
  [128, b] so every layer's matmul consumes the previous layer's output
  directly (contraction dim on partitions), no transposes on device.
- Algebraic rewrites (exact in exact arithmetic):
  * Layer-1 input is identical across T -> its matmul runs once, and the LIF
    recurrence with constant drive c collapses to threshold masks on c:
      s0=[c>=2], s1=[c>=4/3], s2=[c>=8/7]-[c>=4/3]+[c>=2],
      s3=[c>=16/15]-[c>=8/7]+[c>=4/3].
  * Layers 2/3 use a scaled membrane state p_t = 2^(t+1) v_t:
      p += 2^t * c_t ; spike iff p >= 2^(t+1) ; p *= [p < 2^(t+1)]
    and the 2^t factor is folded into the previous layer's spike magnitudes,
    so the "+ c_t" accumulation happens inside PSUM for free.
  * Spikes of layer 3 accumulate as {0,1} counts; the /T mean is folded into
    Wm (host-side Wm.T * 0.25).
- Layer-2/3 matmuls run in fp8(e4m3) with perf_mode=DoubleRow (2 weights per
  PE cell, K=256 per pass): weights are scaled by 64 into e4m3's normal range
  (descale folded into the PSUM-eviction scale, which is free) and spike
  values {1,2,4,8} are exact in fp8. Layer-1/head matmuls are bf16. PSUM
  accumulates f32; LIF state is bf16; outputs f32.
- VectorE tuning: all LIF/mask state lives in PAIR tiles [128, 2, CHUNK]
  matching the DoubleRow rhs layout, so each elementwise op covers two
  128-feature groups at free-size 1024 (halves op count and amortizes the
  per-op overhead). scalar_tensor_tensor runs only at 1x on the DVE, so the
  reset/feat-accumulate fusions are split into tensor_scalar (4x) +
  tensor_tensor (2x) pairs, which are cheaper at this width.
"""

import numpy as np
import ml_dtypes  # noqa: F401

N_CORES = 8
B = 16384
B_CORE = B // N_CORES          # 2048
CHUNK = 512                    # batch columns per pipeline chunk
N_CHUNKS = B_CORE // CHUNK
OBS_DIM, CTX_DIM, HID, ACT = 128, 64, 1024, 32
KT = HID // 128                # 8 partition tiles of the hidden dim
T = 4

_BF16 = ml_dtypes.bfloat16

_compiled = None

# fp8 DoubleRow mode for the layer-2/3 matmuls: weights are scaled by WSCALE
# into e4m3's normal range (descale folded into the PSUM-eviction scale) and
# spikes are emitted as fp8 directly (values {1,2,4,8} are exact). DoubleRow
# packs 2 fp8 weights per PE cell -> K=256 contraction per pass.
FP8 = True
WSCALE = 64.0
KP = KT // 2                   # 4 pair-tiles of 256 features

POOL_CFG = dict(c1p=5, s1p=6, s2p=6, c2p=9, c3p=9, pp=5, featp=5,
                tmpp=6, outp=2, ps=7)


def _build(repeat=1):
    """repeat>1 wraps the compute in a device-side For_i loop (used only for
    slope-based timing; the graded path uses repeat=1, no loop)."""
    from contextlib import nullcontext
    from concourse import bacc, tile
    import concourse.mybir as mybir

    f32 = mybir.dt.float32
    bf16 = mybir.dt.bfloat16
    GE = mybir.AluOpType.is_ge
    LT = mybir.AluOpType.is_lt
    ADD = mybir.AluOpType.add
    MUL = mybir.AluOpType.mult
    IDENT = mybir.ActivationFunctionType.Identity
    TANH = mybir.ActivationFunctionType.Tanh

    nc = bacc.Bacc("TRN2", target_bir_lowering=False, debug=False)

    # ---- DRAM parameters (per-core shards / replicated weights) ----
    fp8 = mybir.dt.float8e4
    obsT_d = nc.dram_tensor("obsT", [OBS_DIM, B_CORE], bf16, kind="ExternalInput")
    ctxT_d = nc.dram_tensor("ctxT", [CTX_DIM, B_CORE], bf16, kind="ExternalInput")
    w1o_d = nc.dram_tensor("w1o", [OBS_DIM, HID], bf16, kind="ExternalInput")
    w1c_d = nc.dram_tensor("w1c", [CTX_DIM, HID], bf16, kind="ExternalInput")
    if FP8:
        w2_d = nc.dram_tensor("w2", [KP * 128, 2, HID], fp8, kind="ExternalInput")
        w3_d = nc.dram_tensor("w3", [KP * 128, 2, HID], fp8, kind="ExternalInput")
    else:
        w2_d = nc.dram_tensor("w2", [HID, HID], bf16, kind="ExternalInput")
        w3_d = nc.dram_tensor("w3", [HID, HID], bf16, kind="ExternalInput")
    wm_d = nc.dram_tensor("wm", [HID, ACT], bf16, kind="ExternalInput")
    b1_d = nc.dram_tensor("b1", [HID, 1], f32, kind="ExternalInput")
    b2s_d = nc.dram_tensor("b2s", [HID, T], f32, kind="ExternalInput")
    b3s_d = nc.dram_tensor("b3s", [HID, T], f32, kind="ExternalInput")
    bm_d = nc.dram_tensor("bm", [ACT, 1], f32, kind="ExternalInput")
    nz_d = nc.dram_tensor("nz", [ACT, 1], f32, kind="ExternalInput")
    om_d = nc.dram_tensor("out_mean", [ACT, B_CORE], f32, kind="ExternalOutput")
    oa_d = nc.dram_tensor("out_act", [ACT, B_CORE], f32, kind="ExternalOutput")

    with tile.TileContext(nc) as tc:
        with (
            tc.tile_pool(name="const", bufs=1) as const,
            tc.tile_pool(name="xp", bufs=1) as xp,
            tc.tile_pool(name="c1p", bufs=POOL_CFG["c1p"]) as c1p,
            tc.tile_pool(name="s1p", bufs=POOL_CFG["s1p"]) as s1p,
            tc.tile_pool(name="s2p", bufs=POOL_CFG["s2p"]) as s2p,
            tc.tile_pool(name="c2p", bufs=POOL_CFG["c2p"]) as c2p,
            tc.tile_pool(name="c3p", bufs=POOL_CFG["c3p"]) as c3p,
            tc.tile_pool(name="pp", bufs=POOL_CFG["pp"]) as pp,
            tc.tile_pool(name="featp", bufs=POOL_CFG["featp"]) as featp,
            tc.tile_pool(name="tmpp", bufs=POOL_CFG["tmpp"]) as tmpp,
            tc.tile_pool(name="outp", bufs=POOL_CFG["outp"]) as outp,
            tc.tile_pool(name="ps", bufs=POOL_CFG["ps"], space="PSUM") as ps_pool,
            tc.tile_pool(name="ps4", bufs=1, space="PSUM") as ps4_pool,
        ):
            # ---- load inputs + constants (x and layer-1 weights first: MM1
            # can start as soon as these land, while w2/w3 stream in) ----
            obsT = xp.tile([OBS_DIM, B_CORE], bf16, tag="obsT")
            ctxT = xp.tile([CTX_DIM, B_CORE], bf16, tag="ctxT")
            # chunk-0 slice first so MM1 starts as early as possible
            nc.sync.dma_start(obsT[:, :CHUNK], obsT_d[:, :CHUNK])
            nc.sync.dma_start(ctxT[:, :CHUNK], ctxT_d[:, :CHUNK])
            nc.sync.dma_start(obsT[:, CHUNK:], obsT_d[:, CHUNK:])
            nc.sync.dma_start(ctxT[:, CHUNK:], ctxT_d[:, CHUNK:])
            w1o = const.tile([OBS_DIM, HID], bf16, tag="w1o")
            nc.sync.dma_start(w1o[:], w1o_d[:])
            w1c = const.tile([CTX_DIM, HID], bf16, tag="w1c")
            nc.sync.dma_start(w1c[:], w1c_d[:])
            b1 = []
            for k in range(KT):
                tb1 = const.tile([128, 1], f32, tag=f"b1_{k}")
                nc.sync.dma_start(tb1[:], b1_d[k * 128:(k + 1) * 128, :])
                b1.append(tb1)
            w2 = []
            w3 = []
            wm = []
            b2s = []
            b3s = []
            if FP8:
                for k in range(KP):
                    t2 = const.tile([128, 2, HID], fp8, tag=f"w2_{k}")
                    nc.sync.dma_start(t2[:], w2_d[k * 128:(k + 1) * 128, :, :])
                    w2.append(t2)
                for k in range(KP):
                    t3 = const.tile([128, 2, HID], fp8, tag=f"w3_{k}")
                    nc.sync.dma_start(t3[:], w3_d[k * 128:(k + 1) * 128, :, :])
                    w3.append(t3)
            else:
                for k in range(KT):
                    t2 = const.tile([128, HID], bf16, tag=f"w2_{k}")
                    nc.sync.dma_start(t2[:], w2_d[k * 128:(k + 1) * 128, :])
                    w2.append(t2)
                for k in range(KT):
                    t3 = const.tile([128, HID], bf16, tag=f"w3_{k}")
                    nc.sync.dma_start(t3[:], w3_d[k * 128:(k + 1) * 128, :])
                    w3.append(t3)
            for k in range(KT):
                tb2 = const.tile([128, T], f32, tag=f"b2_{k}")
                nc.sync.dma_start(tb2[:], b2s_d[k * 128:(k + 1) * 128, :])
                b2s.append(tb2)
                tb3 = const.tile([128, T], f32, tag=f"b3_{k}")
                nc.sync.dma_start(tb3[:], b3s_d[k * 128:(k + 1) * 128, :])
                b3s.append(tb3)
            for k in range(KT):
                tm = const.tile([128, ACT], bf16, tag=f"wm_{k}")
                nc.sync.dma_start(tm[:], wm_d[k * 128:(k + 1) * 128, :])
                wm.append(tm)
            bmv = const.tile([ACT, 1], f32, tag="bm")
            nc.sync.dma_start(bmv[:], bm_d[:])
            nzr = const.tile([ACT, 1], f32, tag="nzr")
            nc.sync.dma_start(nzr[:], nz_d[:])
            nzc = const.tile([ACT, 1], f32, tag="nzc")
            # clip(noise, -0.1, 0.1) = (noise min 0.1) max -0.1
            nc.vector.tensor_scalar(nzc[:], nzr[:], 0.1, -0.1,
                                    mybir.AluOpType.min, mybir.AluOpType.max)

            loop = tc.For_i(0, repeat, 1) if repeat > 1 else nullcontext()
            with loop:
                _kernel_body(nc, tc, mybir, locals())

    nc.compile()
    return nc


def _kernel_body(nc, tc, mybir, env):
    from types import SimpleNamespace
    v = SimpleNamespace(**env)
    f32 = mybir.dt.float32
    bf16 = mybir.dt.bfloat16
    GE = mybir.AluOpType.is_ge
    LT = mybir.AluOpType.is_lt
    ADD = mybir.AluOpType.add
    MUL = mybir.AluOpType.mult
    IDENT = mybir.ActivationFunctionType.Identity
    TANH = mybir.ActivationFunctionType.Tanh
    (w1o, w1c, w2, w3, wm, b1, b2s, b3s, bmv, nzc, obsT, ctxT) = (
        v.w1o, v.w1c, v.w2, v.w3, v.wm, v.b1, v.b2s, v.b3s, v.bmv, v.nzc,
        v.obsT, v.ctxT)
    (c1p, s1p, s2p, c2p, c3p, pp, featp, tmpp, outp, ps_pool, ps4_pool) = (
        v.c1p, v.s1p, v.s2p, v.c2p, v.c3p, v.pp, v.featp, v.tmpp, v.outp,
        v.ps_pool, v.ps4_pool)
    om_d, oa_d = v.om_d, v.oa_d
    fp8 = mybir.dt.float8e4
    assert FP8, "pair-width body assumes the fp8 DoubleRow layout"
    DR = mybir.MatmulPerfMode.DoubleRow

    # All elementwise state lives in PAIR tiles [128, 2, CHUNK]: slot i holds
    # the 128-feature group m = 2*kk + i. This matches the DoubleRow rhs
    # layout exactly, and lets every LIF/mask op process both slots in ONE
    # DVE instruction (free size 2*CHUNK) -- halving DVE op count.
    def pair_tiles(pool, nm, dt):
        return [pool.tile([128, 2, CHUNK], dt, tag=nm, name=f"{nm}p{i}")
                for i in range(KP)]

    if True:
            for ch in range(N_CHUNKS):
                cs = slice(ch * CHUNK, (ch + 1) * CHUNK)

                # ---- layer 1 matmul + closed-form LIF spikes ----
                # Phase A: matmul, evict, and the single-threshold masks for
                # t=0/1 first, so MM2 t-groups 0/1 unblock early while the
                # t=2/3 combines still run.
                c1s = pair_tiles(c1p, "c1", bf16)
                s1P = {0: pair_tiles(s1p, "s1a", fp8),
                       1: pair_tiles(s1p, "s1b", fp8)}
                for m in range(KT):
                    msl = slice(m * 128, (m + 1) * 128)
                    ps = ps_pool.tile([128, CHUNK], f32, tag="ps")
                    nc.tensor.matmul(ps[:], w1o[:, msl], obsT[:, cs],
                                     start=True, stop=False)
                    nc.tensor.matmul(ps[:], w1c[:, msl], ctxT[:, cs],
                                     start=False, stop=True)
                    kk, half = divmod(m, 2)
                    nc.scalar.activation(c1s[kk][:, half, :], ps[:], IDENT,
                                         bias=b1[m][:], scale=1.0)
                for kk in range(KP):
                    # masks (pre-scaled): s1[0]=[c>=2], s1[1]=2*[c>=4/3]
                    nc.vector.tensor_scalar(s1P[0][kk][:], c1s[kk][:], 2.0, None, GE)
                    nc.vector.tensor_scalar(s1P[1][kk][:], c1s[kk][:], 4.0 / 3.0, 2.0, GE, MUL)
                # Phase B: t=2/3 spike combines
                s1P[2] = pair_tiles(s1p, "s1c", fp8)
                s1P[3] = pair_tiles(s1p, "s1d", fp8)
                for kk in range(KP):
                    c1 = c1s[kk]
                    x4 = tmpp.tile([128, 2, CHUNK], bf16, tag="tmp")
                    f8t = tmpp.tile([128, 2, CHUNK], bf16, tag="tmp")
                    nc.vector.tensor_scalar(x4[:], c1[:], 8.0 / 7.0, 4.0, GE, MUL)
                    nc.vector.tensor_scalar(f8t[:], c1[:], 16.0 / 15.0, 8.0, GE, MUL)
                    # f8t := 8*[c>=16/15] - 8*[c>=8/7]  (reads x4 before it
                    # is repurposed below); split STT -> TS + TT (cheaper)
                    xn = tmpp.tile([128, 2, CHUNK], bf16, tag="tmp", name="xn")
                    nc.vector.tensor_scalar(xn[:], x4[:], -2.0, None, MUL)
                    nc.vector.tensor_tensor(f8t[:], f8t[:], xn[:], ADD)
                    # s1[2] = 4*[c>=8/7] - 4*[c>=4/3] + 4*[c>=2]; the -4*m43
                    # term comes from a fresh bf16 mask (TS 4x + TT 2x beats
                    # an fp8-input STT at 1x)
                    cn = tmpp.tile([128, 2, CHUNK], bf16, tag="tmp", name="cn")
                    nc.vector.tensor_scalar(cn[:], c1[:], 4.0 / 3.0, -4.0, GE, MUL)
                    nc.vector.tensor_tensor(x4[:], x4[:], cn[:], ADD)
                    nc.vector.scalar_tensor_tensor(s1P[2][kk][:], s1P[0][kk][:], 4.0, x4[:], MUL, ADD)
                    # s1[3] = 8*[c>=16/15] - 8*[c>=8/7] + 8*[c>=4/3]
                    nc.vector.scalar_tensor_tensor(s1P[3][kk][:], s1P[1][kk][:], 4.0, f8t[:], MUL, ADD)

                # ---- layer 2 matmuls (spikes pre-scaled by 2^t) + evict ----
                # two m-major sweeps: (t0,t1) first since s1[2]/s1[3] arrive
                # later from the phase-B combines
                c2 = {}
                for tpair in ((0, 1), (2, 3)):
                    for m in range(KT):
                        kk, half = divmod(m, 2)
                        for t in tpair:
                            msl = slice(m * 128, (m + 1) * 128)
                            ps = ps_pool.tile([128, CHUNK], f32, tag="ps")
                            for k in range(KP):
                                nc.tensor.matmul(ps[:], w2[k][:, :, msl],
                                                 s1P[t][k][:],
                                                 start=(k == 0), stop=(k == KP - 1),
                                                 perf_mode=DR)
                            if half == 0:
                                c2[(kk, t)] = c2p.tile([128, 2, CHUNK], bf16,
                                                       tag="c2", name=f"c2k{kk}t{t}")
                            nc.scalar.activation(c2[(kk, t)][:, half, :], ps[:], IDENT,
                                                 bias=b2s[m][:, t:t + 1],
                                                 scale=1.0 / WSCALE)

                # ---- layer 2 LIF (scaled state), pair-wide ----
                # STT runs at 1x only (1127 ns @ FD=1024); a TS(4x, 327) +
                # TT(2x, 594) pair is cheaper, so resets are split.
                s2P = {t: pair_tiles(s2p, f"s2{'abcd'[t]}", fp8) for t in range(T)}
                for kk in range(KP):
                    p2 = pp.tile([128, 2, CHUNK], bf16, tag="p2")
                    nc.vector.tensor_scalar(s2P[0][kk][:], c2[(kk, 0)][:], 2.0, None, GE)
                    km = tmpp.tile([128, 2, CHUNK], bf16, tag="km2", bufs=4)
                    nc.vector.tensor_scalar(km[:], c2[(kk, 0)][:], 2.0, None, LT)
                    nc.vector.tensor_tensor(p2[:], c2[(kk, 0)][:], km[:], MUL)
                    for t in range(1, T):
                        thr = float(2 ** (t + 1))
                        sig = float(2 ** t)
                        nc.vector.tensor_tensor(p2[:], p2[:], c2[(kk, t)][:], ADD)
                        nc.vector.tensor_scalar(s2P[t][kk][:], p2[:], thr, sig, GE, MUL)
                        if t < T - 1:
                            km = tmpp.tile([128, 2, CHUNK], bf16, tag="km2", bufs=4)
                            nc.vector.tensor_scalar(km[:], p2[:], thr, None, LT)
                            nc.vector.tensor_tensor(p2[:], p2[:], km[:], MUL)

                # ---- layer 3 matmuls + evict (same two-sweep order) ----
                c3 = {}
                for tpair in ((0, 1), (2, 3)):
                    for m in range(KT):
                        kk, half = divmod(m, 2)
                        for t in tpair:
                            msl = slice(m * 128, (m + 1) * 128)
                            ps = ps_pool.tile([128, CHUNK], f32, tag="ps")
                            for k in range(KP):
                                nc.tensor.matmul(ps[:], w3[k][:, :, msl],
                                                 s2P[t][k][:],
                                                 start=(k == 0), stop=(k == KP - 1),
                                                 perf_mode=DR)
                            if half == 0:
                                c3[(kk, t)] = c3p.tile([128, 2, CHUNK], bf16,
                                                       tag="c3", name=f"c3k{kk}t{t}")
                            nc.scalar.activation(c3[(kk, t)][:, half, :], ps[:], IDENT,
                                                 bias=b3s[m][:, t:t + 1],
                                                 scale=1.0 / WSCALE)

                # ---- layer 3 LIF + spike-count accumulation, pair-wide ----
                feat = {}
                for kk in range(KP):
                    p3 = pp.tile([128, 2, CHUNK], bf16, tag="p3")
                    ft = featp.tile([128, 2, CHUNK], bf16, tag="feat")
                    nc.vector.tensor_scalar(ft[:], c3[(kk, 0)][:], 2.0, None, GE)
                    km = tmpp.tile([128, 2, CHUNK], bf16, tag="km3", bufs=4)
                    nc.vector.tensor_scalar(km[:], c3[(kk, 0)][:], 2.0, None, LT)
                    nc.vector.tensor_tensor(p3[:], c3[(kk, 0)][:], km[:], MUL)
                    for t in range(1, T):
                        thr = float(2 ** (t + 1))
                        nc.vector.tensor_tensor(p3[:], p3[:], c3[(kk, t)][:], ADD)
                        # feat += [p3 >= thr], split as TS mask + TT add
                        sm = tmpp.tile([128, 2, CHUNK], bf16, tag="sm3", bufs=4)
                        nc.vector.tensor_scalar(sm[:], p3[:], thr, None, GE)
                        nc.vector.tensor_tensor(ft[:], ft[:], sm[:], ADD)
                        if t < T - 1:
                            km = tmpp.tile([128, 2, CHUNK], bf16, tag="km3", bufs=4)
                            nc.vector.tensor_scalar(km[:], p3[:], thr, None, LT)
                            nc.vector.tensor_tensor(p3[:], p3[:], km[:], MUL)
                    feat[kk] = ft

                # ---- output head: tanh(feat @ (Wm.T/4) + bm), + clipped noise ----
                ps4 = ps4_pool.tile([ACT, CHUNK], f32, tag="ps4")
                for k in range(KT):
                    nc.tensor.matmul(ps4[:], wm[k][:], feat[k // 2][:, k % 2, :],
                                     start=(k == 0), stop=(k == KT - 1))
                am = outp.tile([ACT, CHUNK], f32, tag="am")
                nc.scalar.activation(am[:], ps4[:], TANH, bias=bmv[:], scale=1.0)
                aa = outp.tile([ACT, CHUNK], f32, tag="aa")
                nc.vector.tensor_scalar(aa[:], am[:], nzc[:], None, ADD)
                nc.sync.dma_start(om_d[:, cs], am[:])
                nc.sync.dma_start(oa_d[:, cs], aa[:])


def _get_compiled(repeat=1):
    global _compiled
    if _compiled is None:
        _compiled = _build(repeat=repeat)
    return _compiled


LAST_RESULTS = None
_pjrt_fns = {}


def _prep_in_maps(obs, context, noise, W1, b1, W2, b2, W3, b3, Wm, bm):
    # host-side prep: transpose to [feature, batch], cast to bf16, fold scales
    obsT = np.ascontiguousarray(obs.T.astype(_BF16))              # [128, B]
    ctxT = np.ascontiguousarray(context.T.astype(_BF16))          # [64, B]
    w1o = np.ascontiguousarray(W1[:, :OBS_DIM].T.astype(_BF16))   # [128, HID]
    w1c = np.ascontiguousarray(W1[:, OBS_DIM:].T.astype(_BF16))   # [64, HID]
    if FP8:
        import concourse.mybir as mybir
        f8np = mybir.dt.np(mybir.dt.float8e4)

        def pack_dr(W):
            # [HID(h), HID(o)] -> [KP*128, 2, HID]: h = kk*256 + i*128 + p
            wt = (W.T * WSCALE).reshape(KP, 2, 128, HID)
            return np.ascontiguousarray(
                wt.transpose(0, 2, 1, 3).reshape(KP * 128, 2, HID).astype(f8np))

        w2 = pack_dr(W2)
        w3 = pack_dr(W3)
    else:
        w2 = np.ascontiguousarray(W2.T.astype(_BF16))             # [HID, HID]
        w3 = np.ascontiguousarray(W3.T.astype(_BF16))             # [HID, HID]
    wm = np.ascontiguousarray((Wm.T * 0.25).astype(_BF16))        # [HID, ACT]
    scales = (2.0 ** np.arange(T, dtype=np.float32))              # [1,2,4,8]
    b1c = np.ascontiguousarray(b1.astype(np.float32).reshape(HID, 1))
    b2s = np.ascontiguousarray(b2.astype(np.float32)[:, None] * scales[None, :])
    b3s = np.ascontiguousarray(b3.astype(np.float32)[:, None] * scales[None, :])
    bmc = np.ascontiguousarray(bm.astype(np.float32).reshape(ACT, 1))
    nz = np.ascontiguousarray(noise.astype(np.float32).reshape(ACT, 1))

    shared = {"w1o": w1o, "w1c": w1c, "w2": w2, "w3": w3, "wm": wm,
              "b1": b1c, "b2s": b2s, "b3s": b3s, "bm": bmc, "nz": nz}
    in_maps = []
    for c in range(N_CORES):
        bs = slice(c * B_CORE, (c + 1) * B_CORE)
        m = dict(shared)
        m["obsT"] = np.ascontiguousarray(obsT[:, bs])
        m["ctxT"] = np.ascontiguousarray(ctxT[:, bs])
        in_maps.append(m)
    return in_maps


def _get_pjrt_fn(nc):
    """Replicates bass2jax.run_bass_via_pjrt's sharded jit, cached so repeat
    calls reuse the compiled executable. Returns (fn, in_names, out_names,
    out_avals, n_params)."""
    if id(nc) in _pjrt_fns:
        return _pjrt_fns[id(nc)]
    import jax
    import concourse.mybir as mybir
    from jax.sharding import Mesh, PartitionSpec
    from jax.experimental.shard_map import shard_map
    from concourse.bass2jax import (install_neuronx_cc_hook, _bass_exec_p,
                                    partition_id_tensor)

    install_neuronx_cc_hook()
    assert nc.dbg_addr is None
    partition_name = (nc.partition_id_tensor.name
                      if nc.partition_id_tensor else None)

    in_names, out_names, out_avals = [], [], []
    for alloc in nc.m.functions[0].allocations:
        if not isinstance(alloc, mybir.MemoryLocationSet):
            continue
        name = alloc.memorylocations[0].name
        if alloc.kind == "ExternalInput":
            if name != partition_name:
                in_names.append(name)
        elif alloc.kind == "ExternalOutput":
            shape = tuple(alloc.tensor_shape)
            dtype = mybir.dt.np(alloc.dtype)
            out_names.append(name)
            out_avals.append(jax.core.ShapedArray(shape, dtype))
    n_params = len(in_names)
    n_outs = len(out_names)
    all_names = in_names + out_names
    if partition_name is not None:
        all_names = all_names + [partition_name]

    def _body(*args):
        operands = list(args)
        if partition_name is not None:
            operands.append(partition_id_tensor())
        outs = _bass_exec_p.bind(
            *operands,
            out_avals=tuple(out_avals),
            in_names=tuple(all_names),
            out_names=tuple(out_names),
            lowering_input_output_aliases=(),
            sim_require_finite=True,
            sim_require_nnan=True,
            nc=nc,
        )
        return tuple(outs)

    devices = jax.devices()[:N_CORES]
    mesh = Mesh(np.asarray(devices), ("core",))
    in_specs = (PartitionSpec("core"),) * (n_params + n_outs)
    out_specs = (PartitionSpec("core"),) * n_outs
    fn = jax.jit(
        shard_map(_body, mesh=mesh, in_specs=in_specs, out_specs=out_specs,
                  check_rep=False),
        donate_argnums=tuple(range(n_params, n_params + n_outs)),
        keep_unused=True,
    )
    _pjrt_fns[id(nc)] = (fn, mesh, in_names, out_names, out_avals, n_params)
    return _pjrt_fns[id(nc)]


def _run(nc, in_maps, time_iters=0):
    """Run the compiled graph on 8 cores via PJRT. Returns (per-core results,
    best wall ns over time_iters extra timed runs or None)."""
    import jax
    import time as _time
    from jax.sharding import NamedSharding, PartitionSpec

    fn, mesh, in_names, out_names, out_avals, n_params = _get_pjrt_fn(nc)
    sh = NamedSharding(mesh, PartitionSpec("core"))

    concat_in = [
        np.concatenate([np.asarray(in_maps[c][name]) for c in range(N_CORES)], axis=0)
        for name in in_names
    ]
    dev_in = [jax.device_put(a, sh) for a in concat_in]

    def make_zeros():
        return [
            jax.device_put(
                np.zeros((N_CORES * av.shape[0], *av.shape[1:]), av.dtype), sh)
            for av in out_avals
        ]

    out_arrs = fn(*dev_in, *make_zeros())
    jax.block_until_ready(out_arrs)

    best_ns = None
    for _ in range(time_iters):
        zs = make_zeros()
        jax.block_until_ready(zs)
        t0 = _time.perf_counter()
        o = fn(*dev_in, *zs)
        jax.block_until_ready(o)
        dt = (_time.perf_counter() - t0) * 1e9
        best_ns = dt if best_ns is None else min(best_ns, dt)

    results = [
        {name: np.asarray(out_arrs[i]).reshape(N_CORES, *out_avals[i].shape)[c]
         for i, name in enumerate(out_names)}
        for c in range(N_CORES)
    ]
    return results, best_ns


def kernel(obs, context, noise, W1, b1, W2, b2, W3, b3, Wm, bm):
    obs, context, noise, W1, b1, W2, b2, W3, b3, Wm, bm = (
        np.asarray(a, dtype=np.float32)
        for a in (obs, context, noise, W1, b1, W2, b2, W3, b3, Wm, bm))
    nc = _get_compiled()
    in_maps = _prep_in_maps(obs, context, noise, W1, b1, W2, b2, W3, b3, Wm, bm)
    results, _ = _run(nc, in_maps)
    am = np.concatenate([results[c]["out_mean"] for c in range(N_CORES)], axis=1)
    aa = np.concatenate([results[c]["out_act"] for c in range(N_CORES)], axis=1)
    action_mean = np.ascontiguousarray(am.T).astype(np.float32)
    action = np.ascontiguousarray(aa.T).astype(np.float32)
    return (action_mean, action)


def bench(inputs, iters=20):
    """Returns best wall-clock ns for one 8-core dispatch."""
    nc = _get_compiled()
    in_maps = _prep_in_maps(**inputs)
    _, best_ns = _run(nc, in_maps, time_iters=iters)
    return best_ns


if __name__ == "__main__":
    nc = _get_compiled()
    print("compiled OK")
